# revision 8
# baseline (speedup 1.0000x reference)
"""Trainium2 Bass kernel for nn_Decoder (Bahdanau-attention LSTM decoder step).

Strategy (8 NeuronCores, no cross-core collectives):
  Kernel 1 — data-parallel over batch (8 batches/core):
    * energy = tanh(encT @ W2.T + u) computed as energyT [h', s] tiles on PE
      (fp16 inputs, f32 PSUM accumulation)
    * scores via PE matmul with v as the stationary operand
    * softmax on-chip (scores gathered to [8,1024] batch-per-partition layout
      via one SBUF->SBUF DMA)
    * context via PE matmul with transposed softmax weights (PE-transpose)
    * full LSTM cell for the core's own batches (gate matmul streams the full
      fused [W_ih|W_hh|bias] matrix; bias folded in via a ones row)
    outputs: h_new [8,1024] f32, c_new [8,1024] f32, context [8,1024] f16
  Host gathers h/context (tiny) and feeds kernel 2.
  Kernel 2 — tensor-parallel over vocab (4000 rows/core):
    logits[64, vslice] = [h_new | context | 1] @ [W_fc | b_fc].T in fp16.

All big operands are cast to fp16 on host (halves HBM traffic, full-rate PE);
all accumulation is f32.  Measured end-to-end error vs the f32 reference is
~4.6e-3 scale-relative absmax (fp16 quantization of the attention matmul
dominates; bf16 would be ~3.1e-2).
"""

import numpy as np

V, E, H, B, S = 32000, 512, 1024, 64, 1024
NCORES = 8
BL = B // NCORES          # 8 local batches per core
VSL = V // NCORES         # 4000 vocab rows per core
VSLP = 4096               # padded vocab slice
KG = 21                   # gate matmul k-chunks: 4 emb + 8 ctx + 8 hh + 1 bias
KF = 17                   # fc matmul k-chunks: 8 h + 8 ctx + 1 bias

_PROGS = {}


def _build_kernel1():
    from contextlib import ExitStack
    import concourse.bacc as bacc
    import concourse.tile as tile
    import concourse.mybir as mybir

    F32, F16 = mybir.dt.float32, mybir.dt.float16
    AF = mybir.ActivationFunctionType

    nc = bacc.Bacc("TRN2", target_bir_lowering=False, debug=False, num_devices=NCORES)

    encT = nc.dram_tensor("encT", [BL, H, S], F16, kind="ExternalInput")
    enc = nc.dram_tensor("enc", [BL, S, H], F16, kind="ExternalInput")
    w2t = nc.dram_tensor("w2t", [H, H], F16, kind="ExternalInput")
    w1t = nc.dram_tensor("w1t", [H, H], F16, kind="ExternalInput")
    vt = nc.dram_tensor("vt", [128, 8], F16, kind="ExternalInput")
    attnbT = nc.dram_tensor("attnbT", [128, 8], F32, kind="ExternalInput")
    xgate = nc.dram_tensor("xgate", [128, 13, BL], F16, kind="ExternalInput")
    wcatT = nc.dram_tensor("wcatT", [KG * 128, 4 * H], F16, kind="ExternalInput")
    clast = nc.dram_tensor("clast", [BL, H], F32, kind="ExternalInput")
    eye = nc.dram_tensor("eye", [128, 128], F16, kind="ExternalInput")

    h_out = nc.dram_tensor("h_out", [BL, H], F32, kind="ExternalOutput")
    c_out = nc.dram_tensor("c_out", [BL, H], F32, kind="ExternalOutput")
    ctx_out = nc.dram_tensor("ctx_out", [BL, H], F16, kind="ExternalOutput")
    scores_dram = nc.dram_tensor("scores_dram", [2 * BL, 512], F32)
    ctx_dram = nc.dram_tensor("ctx_dram", [BL, H], F16)

    with tile.TileContext(nc) as tc, ExitStack() as ctx:
        const = ctx.enter_context(tc.tile_pool(name="const", bufs=1))
        encT_p = ctx.enter_context(tc.tile_pool(name="encT_p", bufs=2))
        enc_p = ctx.enter_context(tc.tile_pool(name="enc_p", bufs=2))
        tanh_p = ctx.enter_context(tc.tile_pool(name="tanh_p", bufs=4))
        wcat_p = ctx.enter_context(tc.tile_pool(name="wcat_p", bufs=3))
        w1_p = ctx.enter_context(tc.tile_pool(name="w1_p", bufs=2))
        row_p = ctx.enter_context(tc.tile_pool(name="row_p", bufs=3))
        misc = ctx.enter_context(tc.tile_pool(name="misc", bufs=1))
        lstm_p = ctx.enter_context(tc.tile_pool(name="lstm_p", bufs=1))
        psum = ctx.enter_context(tc.tile_pool(name="psum", bufs=8, space="PSUM"))

        # ---- constant loads ----
        w2t_sb = const.tile([128, 8, H], F16, tag="w2t_sb")
        nc.sync.dma_start(out=w2t_sb[:], in_=w2t.ap().rearrange("(hc p) n -> p hc n", p=128))
        vt_sb = const.tile([128, 8], F16, tag="vt_sb")
        nc.sync.dma_start(out=vt_sb[:], in_=vt.ap())
        attnb_sb = const.tile([128, 8], F32, tag="attnb_sb")
        nc.sync.dma_start(out=attnb_sb[:], in_=attnbT.ap())
        xg_sb = const.tile([128, 13, BL], F16, tag="xg_sb")
        nc.sync.dma_start(out=xg_sb[:], in_=xgate.ap())
        eye_sb = const.tile([128, 128], F16, tag="eye_sb")
        nc.sync.dma_start(out=eye_sb[:], in_=eye.ap())
        clast_sb = const.tile([BL, H], F32, tag="clast_sb")
        nc.sync.dma_start(out=clast_sb[:], in_=clast.ap())

        # ---- phase U: uT[h', b] = W1 @ h_last.T + attn_b ----
        u_sb = misc.tile([128, 8, BL], F32, tag="u_sb")
        pus = [psum.tile([128, BL], F32, tag="ps", name=f"pu{i}") for i in range(8)]
        for hc in range(8):
            w1c = w1_p.tile([128, H], F16, tag="w1c")
            nc.sync.dma_start(out=w1c[:], in_=w1t.ap()[hc * 128:(hc + 1) * 128, :])
            for t in range(8):
                nc.tensor.matmul(pus[t][:], lhsT=w1c[:, t * 128:(t + 1) * 128],
                                 rhs=xg_sb[:, 4 + hc, :], start=(hc == 0), stop=(hc == 7))
        for t in range(8):
            nc.scalar.activation(u_sb[:, t, :], pus[t][:], AF.Identity, bias=attnb_sb[:, t:t + 1])

        # ---- phase A: energyT tiles + scores ----
        for b in range(BL):
            et = encT_p.tile([128, 8, S], F16, tag="et")
            nc.sync.dma_start(out=et[:], in_=encT.ap()[b].rearrange("(hc p) s -> p hc s", p=128))
            for sblk in range(2):
                ps_s = psum.tile([1, 512], F32, tag="ps")
                for t in range(8):
                    pe = psum.tile([128, 512], F32, tag="ps")
                    for hc in range(8):
                        nc.tensor.matmul(pe[:], lhsT=w2t_sb[:, hc, t * 128:(t + 1) * 128],
                                         rhs=et[:, hc, sblk * 512:(sblk + 1) * 512],
                                         start=(hc == 0), stop=(hc == 7))
                    th = tanh_p.tile([128, 512], F16, tag="th")
                    nc.scalar.activation(th[:], pe[:], AF.Tanh, bias=u_sb[:, t, b:b + 1])
                    nc.tensor.matmul(ps_s[:], lhsT=vt_sb[:, t:t + 1], rhs=th[:],
                                     start=(t == 0), stop=(t == 7))
                srow = row_p.tile([1, 512], F32, tag="srow")
                nc.vector.tensor_copy(srow[:], ps_s[:])
                nc.gpsimd.dma_start(out=scores_dram.ap()[b * 2 + sblk, :], in_=srow[:])

        # ---- softmax (batch-per-partition layout) ----
        scores_sb = misc.tile([8, 1024], F32, tag="scores_sb")
        nc.gpsimd.dma_start(out=scores_sb[:], in_=scores_dram.ap().rearrange("(b k) s -> b (k s)", b=8))
        negmax = misc.tile([8, 1], F32, tag="negmax")
        nc.vector.tensor_reduce(negmax[:], scores_sb[:], axis=mybir.AxisListType.X,
                                op=mybir.AluOpType.max, negate=True)
        exp_sb = misc.tile([8, 1024], F32, tag="exp_sb")
        sums = misc.tile([8, 1], F32, tag="sums")
        nc.scalar.activation(exp_sb[:], scores_sb[:], AF.Exp, bias=negmax[:], accum_out=sums[:])
        recip = misc.tile([8, 1], F32, tag="recip")
        nc.vector.reciprocal(recip[:], sums[:])
        w16 = misc.tile([16, 1024], F16, tag="w16")
        nc.vector.memset(w16[:], 0.0)
        nc.vector.tensor_scalar_mul(w16[0:8, :], exp_sb[:], recip[:])
        wT_sb = misc.tile([128, 8, 16], F16, tag="wT_sb")
        for j in range(8):
            pw = psum.tile([128, 16], F16, tag="ps")
            nc.tensor.transpose(pw[:], w16[0:16, j * 128:(j + 1) * 128], eye_sb[0:16, 0:16])
            nc.vector.tensor_copy(wT_sb[:, j, :], pw[:])

        # ---- context (normalized weights @ enc) ----
        for b in range(BL):
            en = enc_p.tile([128, 8, H], F16, tag="en")
            nc.gpsimd.dma_start(out=en[:], in_=enc.ap()[b].rearrange("(sc p) h -> p sc h", p=128))
            for nblk in range(2):
                pc = psum.tile([1, 512], F32, tag="ps")
                for sc in range(8):
                    nc.tensor.matmul(pc[:], lhsT=wT_sb[:, sc, b:b + 1],
                                     rhs=en[:, sc, nblk * 512:(nblk + 1) * 512],
                                     start=(sc == 0), stop=(sc == 7))
                crow = row_p.tile([1, 512], F16, tag="crow")
                nc.vector.tensor_copy(crow[:], pc[:])
                nc.gpsimd.dma_start(out=ctx_dram.ap()[b, nblk * 512:(nblk + 1) * 512], in_=crow[:])
        ctxl = misc.tile([16, 1024], F16, tag="ctxl")
        nc.vector.memset(ctxl[:], 0.0)
        nc.gpsimd.dma_start(out=ctxl[0:8, :], in_=ctx_dram.ap())
        nc.sync.dma_start(out=ctx_out[:], in_=ctxl[0:8, :])
        ctxT_sb = misc.tile([128, 8, 16], F16, tag="ctxT_sb")
        for j in range(8):
            pw = psum.tile([128, 16], F16, tag="ps")
            nc.tensor.transpose(pw[:], ctxl[0:16, j * 128:(j + 1) * 128], eye_sb[0:16, 0:16])
            nc.vector.tensor_copy(ctxT_sb[:, j, :], pw[:])

        # ---- gates: [8, 4096] = x @ [W_ih|W_hh|b].T  (k streamed, 8 psum banks) ----
        gpsums = [psum.tile([BL, 512], F32, tag="ps", name=f"pg{i}") for i in range(8)]
        for kc in range(KG):
            wblk = wcat_p.tile([128, 4 * H], F16, tag="wblk")
            nc.sync.dma_start(out=wblk[:], in_=wcatT.ap()[kc * 128:(kc + 1) * 128, :])
            if kc < 4:
                lhs = xg_sb[:, kc, :]
            elif kc < 12:
                lhs = ctxT_sb[:, kc - 4, 0:BL]
            else:
                lhs = xg_sb[:, kc - 8, :]
            for rblk in range(8):
                nc.tensor.matmul(gpsums[rblk][:], lhsT=lhs,
                                 rhs=wblk[:, rblk * 512:(rblk + 1) * 512],
                                 start=(kc == 0), stop=(kc == KG - 1))

        # ---- LSTM cell elementwise ----
        gate_sb = misc.tile([BL, 8, 512], F32, tag="gate_sb")
        funcs = [AF.Sigmoid, AF.Sigmoid, AF.Sigmoid, AF.Sigmoid,
                 AF.Tanh, AF.Tanh, AF.Sigmoid, AF.Sigmoid]
        for rblk in range(8):
            nc.scalar.activation(gate_sb[:, rblk, :], gpsums[rblk][:], funcs[rblk])
        c_new_sb = lstm_p.tile([BL, H], F32, tag="c_new_sb")
        h_new_sb = lstm_p.tile([BL, H], F32, tag="h_new_sb")
        for half in range(2):
            sl = slice(half * 512, (half + 1) * 512)
            i_ap = gate_sb[:, 0 + half, :]
            f_ap = gate_sb[:, 2 + half, :]
            g_ap = gate_sb[:, 4 + half, :]
            o_ap = gate_sb[:, 6 + half, :]
            t1 = lstm_p.tile([BL, 512], F32, tag="t1")
            t2 = lstm_p.tile([BL, 512], F32, tag="t2")
            nc.vector.tensor_mul(t1[:], f_ap, clast_sb[:, sl])
            nc.vector.tensor_mul(t2[:], i_ap, g_ap)
            nc.vector.tensor_add(c_new_sb[:, sl], t1[:], t2[:])
            tch = lstm_p.tile([BL, 512], F32, tag="tch")
            nc.scalar.activation(tch[:], c_new_sb[:, sl], AF.Tanh)
            nc.vector.tensor_mul(h_new_sb[:, sl], o_ap, tch[:])
        nc.sync.dma_start(out=c_out[:], in_=c_new_sb[:])
        nc.sync.dma_start(out=h_out[:], in_=h_new_sb[:])

    nc.finalize()
    return nc


def _build_kernel2():
    from contextlib import ExitStack
    import concourse.bacc as bacc
    import concourse.tile as tile
    import concourse.mybir as mybir

    F32, F16 = mybir.dt.float32, mybir.dt.float16

    nc = bacc.Bacc("TRN2", target_bir_lowering=False, debug=False, num_devices=NCORES)

    hcT = nc.dram_tensor("hcT", [128, KF, B], F16, kind="ExternalInput")
    wfcaT = nc.dram_tensor("wfcaT", [KF * 128, VSLP], F16, kind="ExternalInput")
    logits = nc.dram_tensor("logits", [B, VSLP], F32, kind="ExternalOutput")

    with tile.TileContext(nc) as tc, ExitStack() as ctx:
        const = ctx.enter_context(tc.tile_pool(name="const", bufs=1))
        wf_p = ctx.enter_context(tc.tile_pool(name="wf_p", bufs=6))
        out_p = ctx.enter_context(tc.tile_pool(name="out_p", bufs=1))
        psum = ctx.enter_context(tc.tile_pool(name="psum", bufs=8, space="PSUM"))

        hc_sb = const.tile([128, KF, B], F16, tag="hc_sb")
        nc.sync.dma_start(out=hc_sb[:], in_=hcT.ap())
        psums = [psum.tile([B, 512], F32, tag="ps", name=f"pl{i}") for i in range(8)]
        for kc in range(KF):
            wb = wf_p.tile([128, VSLP], F16, tag="wb")
            nc.sync.dma_start(out=wb[:], in_=wfcaT.ap()[kc * 128:(kc + 1) * 128, :])
            for vb in range(8):
                nc.tensor.matmul(psums[vb][:], lhsT=hc_sb[:, kc, :],
                                 rhs=wb[:, vb * 512:(vb + 1) * 512],
                                 start=(kc == 0), stop=(kc == KF - 1))
        lg = out_p.tile([B, 8, 512], F32, tag="lg")
        for vb in range(8):
            nc.vector.tensor_copy(lg[:, vb, :], psums[vb][:])
        nc.sync.dma_start(out=logits[:], in_=lg[:].rearrange("p a b -> p (a b)"))

    nc.finalize()
    return nc


def _get_progs():
    if "k1" not in _PROGS:
        _PROGS["k1"] = _build_kernel1()
        _PROGS["k2"] = _build_kernel2()
    return _PROGS["k1"], _PROGS["k2"]


def _prep_inputs(inputs):
    """Host-side sharding + fp16 casting. Returns (in_maps1 list, static dict)."""
    f16 = np.float16
    idx = np.asarray(inputs["input"]).astype(np.int64)
    hidden = np.asarray(inputs["hidden"], dtype=np.float32)
    cell = np.asarray(inputs["cell"], dtype=np.float32)
    enc = np.asarray(inputs["encoder_outputs"], dtype=np.float32)
    emb = np.asarray(inputs["emb"], dtype=np.float32)
    attn_W = np.asarray(inputs["attn_W"], dtype=np.float32)
    attn_b = np.asarray(inputs["attn_b"], dtype=np.float32)
    v = np.asarray(inputs["v"], dtype=np.float32)
    W_ih = np.asarray(inputs["W_ih"], dtype=np.float32)
    W_hh = np.asarray(inputs["W_hh"], dtype=np.float32)
    b_ih = np.asarray(inputs["b_ih"], dtype=np.float32)
    b_hh = np.asarray(inputs["b_hh"], dtype=np.float32)
    W_fc = np.asarray(inputs["W_fc"], dtype=np.float32)
    b_fc = np.asarray(inputs["b_fc"], dtype=np.float32)

    h_last = hidden[-1]                    # [B, H]
    c_last = cell[-1]                      # [B, H]
    embedded = emb[idx]                    # [B, E]

    enc16 = enc.astype(f16)                                   # [B, S, H]
    encT16 = np.ascontiguousarray(enc16.transpose(0, 2, 1))   # [B, H, S]

    w2t = np.ascontiguousarray(attn_W[:, H:].T).astype(f16)   # [H, H]
    w1t = np.ascontiguousarray(attn_W[:, :H].T).astype(f16)
    vt = np.ascontiguousarray(v.reshape(8, 128).T).astype(f16)          # [128, 8]
    attnbT = np.ascontiguousarray(attn_b.reshape(8, 128).T).astype(np.float32)

    wcatT = np.zeros([KG * 128, 4 * H], dtype=f16)
    wcatT[0:E] = W_ih[:, :E].T.astype(f16)
    wcatT[E:E + H] = W_ih[:, E:].T.astype(f16)
    wcatT[E + H:E + 2 * H] = W_hh.T.astype(f16)
    wcatT[E + 2 * H] = (b_ih + b_hh).astype(f16)

    eye = np.eye(128, dtype=f16)

    h_lastT16 = h_last.T.astype(f16)       # [H, B]
    embT16 = embedded.T.astype(f16)        # [E, B]

    in_maps1 = []
    for c in range(NCORES):
        bs = slice(c * BL, (c + 1) * BL)
        xg = np.zeros([128, 13, BL], dtype=f16)
        xg[:, 0:4, :] = embT16[:, bs].reshape(4, 128, BL).transpose(1, 0, 2)
        xg[:, 4:12, :] = h_lastT16[:, bs].reshape(8, 128, BL).transpose(1, 0, 2)
        xg[0, 12, :] = 1.0
        in_maps1.append({
            "encT": encT16[bs], "enc": enc16[bs],
            "w2t": w2t, "w1t": w1t, "vt": vt, "attnbT": attnbT,
            "xgate": xg, "wcatT": wcatT,
            "clast": np.ascontiguousarray(c_last[bs]),
            "eye": eye,
        })

    # kernel-2 static weights
    wfcas = []
    for c in range(NCORES):
        vs = slice(c * VSL, (c + 1) * VSL)
        wf = np.zeros([KF * 128, VSLP], dtype=f16)
        wf[0:2 * H, 0:VSL] = W_fc[vs].T.astype(f16)
        wf[2 * H, 0:VSL] = b_fc[vs].astype(f16)
        wfcas.append(wf)
    return in_maps1, wfcas


def _hcT_from_k1(res1):
    f16 = np.float16
    h_full = np.concatenate([res1[c]["h_out"] for c in range(NCORES)], axis=0)   # [B, H] f32
    c_full = np.concatenate([res1[c]["c_out"] for c in range(NCORES)], axis=0)   # [B, H] f32
    ctx_full = np.concatenate([res1[c]["ctx_out"] for c in range(NCORES)], axis=0)  # [B, H] f16
    hT = h_full.T.astype(f16)              # [H, B]
    cT = ctx_full.T                        # [H, B] f16
    hcT = np.zeros([128, KF, B], dtype=f16)
    hcT[:, 0:8, :] = hT.reshape(8, 128, B).transpose(1, 0, 2)
    hcT[:, 8:16, :] = cT.reshape(8, 128, B).transpose(1, 0, 2)
    hcT[0, 16, :] = 1.0
    return hcT, h_full, c_full


def kernel(**inputs):
    from concourse.bass_utils import run_bass_kernel_spmd

    nc1, nc2 = _get_progs()
    in_maps1, wfcas = _prep_inputs(inputs)
    core_ids = list(range(NCORES))

    res1 = run_bass_kernel_spmd(nc1, in_maps1, core_ids).results
    hcT, h_full, c_full = _hcT_from_k1(res1)

    in_maps2 = [{"hcT": hcT, "wfcaT": wfcas[c]} for c in range(NCORES)]
    res2 = run_bass_kernel_spmd(nc2, in_maps2, core_ids).results

    prediction = np.concatenate([res2[c]["logits"][:, :VSL] for c in range(NCORES)], axis=1)
    return (prediction.astype(np.float32), h_full[None], c_full[None])
